# revision 1
# baseline (speedup 1.0000x reference)
"""Grouped Conv2d (512 groups, 2->2 ch/group, 3x3 VALID) on 8 trn2 NeuronCores.

Strategy:
  - Shard the 512 groups across 8 cores: 64 groups = 128 channels per core,
    which exactly fills the 128 SBUF partitions. Fully independent (no
    collectives); batch stays whole on every core.
  - On-device compute: for each 3x3 tap (kh,kw) build a 128x128
    block-diagonal weight matrix (64 blocks of 2x2) host-side; the grouped
    conv then becomes 9 accumulating PE matmuls per output tile:
        psum[oc, i, j] += W_tap[ic, oc]^T . x[ic, i+kh, j+kw]
    run in float32r (full-rate fp32 on the PE for moving dim >= 256).
  - Output rows are produced in 6 chunks of 9 rows (9*54 = 486 <= 512 fp32
    = one PSUM bank), evicted PSUM->SBUF on alternating scalar/vector
    engines, and DMA'd back per batch.
"""

import sys

import numpy as np

for _p in ("/opt/trn_rl_repo",):
    if _p not in sys.path:
        sys.path.insert(0, _p)

import concourse.bacc as bacc
import concourse.bass as bass
import concourse.tile as tile
from concourse import mybir
from concourse.bass_utils import run_bass_kernel_spmd

N_CORES = 8
B, C, H, W = 16, 1024, 56, 56
KH = KW = 3
HO, WO = H - KH + 1, W - KW + 1  # 54, 54
CPC = C // N_CORES  # 128 channels (64 groups) per core
ROWS_PER_CHUNK = 9  # 9*54 = 486 fp32 <= 512 (one PSUM bank)
N_CHUNKS = HO // ROWS_PER_CHUNK  # 6

_NC_CACHE = {}


def _build_program(repeats=1):
    nc = bacc.Bacc(
        "TRN2", target_bir_lowering=False, debug=False, num_devices=N_CORES
    )
    f32 = mybir.dt.float32
    f32r = mybir.dt.float32r

    x_d = nc.declare_dram_parameter("x", [B, CPC, H, W], f32, isOutput=False)
    wm_d = nc.declare_dram_parameter(
        "wm", [CPC, KH * KW, CPC], f32, isOutput=False
    )
    y_d = nc.declare_dram_parameter("y", [B, CPC, HO, WO], f32, isOutput=True)

    with tile.TileContext(nc) as tc:
        with (
            tc.tile_pool(name="wpool", bufs=1) as wpool,
            tc.tile_pool(name="xpool", bufs=4) as xpool,
            tc.tile_pool(name="opool", bufs=3) as opool,
            tc.tile_pool(name="psum", bufs=7, space="PSUM") as ppool,
            tc.tile_pool(name="scratch", bufs=1, space="PSUM") as spool,
        ):
            wt = wpool.tile([CPC, KH * KW, CPC], f32r)
            nc.sync.dma_start(out=wt[:], in_=wm_d[:].bitcast(f32r))

            # The fused f32r matmul (LDW+MM) supports only ONE semaphore
            # wait; Tile would otherwise put {wt-DMA, xt-DMA} (2 waits) on
            # the first matmul of each batch. These "sync" matmuls absorb
            # the DMA waits; PE program order covers the rest.
            scr = spool.tile([CPC, 512], f32)
            nc.tensor.matmul(
                scr[:, :2], lhsT=wt[:, 0, :], rhs=wt[:, 0, :2],
                start=True, stop=True,
            )
            # Dummy matmuls keep PE busy during the initial x DMA fill so
            # the HAM clock gate ramps to 2.4 GHz before real work arrives.
            for _ in range(16):
                nc.tensor.matmul(
                    scr[:, :256], lhsT=wt[:, 0, :], rhs=wt[:, 0:2, :],
                    start=True, stop=True,
                )

            def body():
                _emit_batches(nc, tc, xpool, opool, ppool, x_d, y_d, wt, scr)

            if repeats == 1:
                body()
            else:
                with tc.For_i(0, repeats):
                    body()
    nc.compile()
    return nc


def _emit_batches(nc, tc, xpool, opool, ppool, x_d, y_d, wt, scr):
    f32 = mybir.dt.float32
    f32r = mybir.dt.float32r
    HSPLIT = 30  # rows 0..29 cover chunks 0-2; rows 30..55 cover chunks 3-5
    OSPLIT = 3 * ROWS_PER_CHUNK  # first 3 output chunks ship early
    for n in range(B):
                xt = xpool.tile([CPC, H, W], f32r)
                nc.sync.dma_start(
                    out=xt[:, :HSPLIT, :], in_=x_d[n, :, :HSPLIT, :].bitcast(f32r)
                )
                nc.sync.dma_start(
                    out=xt[:, HSPLIT:, :], in_=x_d[n, :, HSPLIT:, :].bitcast(f32r)
                )
                nc.tensor.matmul(
                    scr[:, :2], lhsT=wt[:, 0, :], rhs=xt[:, 0, :2],
                    start=True, stop=True,
                )
                nc.tensor.matmul(
                    scr[:, :2], lhsT=wt[:, 0, :], rhs=xt[:, H - 1, :2],
                    start=True, stop=True,
                )
                ot = opool.tile([CPC, HO, WO], f32)
                for c in range(N_CHUNKS):
                    r0 = c * ROWS_PER_CHUNK
                    pt = ppool.tile([CPC, ROWS_PER_CHUNK, WO], f32)
                    t = 0
                    for kh in range(KH):
                        for kw in range(KW):
                            nc.tensor.matmul(
                                pt[:],
                                lhsT=wt[:, t, :],
                                rhs=xt[
                                    :,
                                    r0 + kh : r0 + kh + ROWS_PER_CHUNK,
                                    kw : kw + WO,
                                ],
                                start=(t == 0),
                                stop=(t == KH * KW - 1),
                            )
                            t += 1
                    dst = ot[:, r0 : r0 + ROWS_PER_CHUNK, :]
                    if c % 2 == 0:
                        nc.scalar.activation(
                            dst, pt[:], mybir.ActivationFunctionType.Copy
                        )
                    else:
                        nc.vector.tensor_copy(dst, pt[:])
                    if c == 2:
                        nc.sync.dma_start(
                            out=y_d[n, :, :OSPLIT, :], in_=ot[:, :OSPLIT, :]
                        )
                nc.sync.dma_start(
                    out=y_d[n, :, OSPLIT:, :], in_=ot[:, OSPLIT:, :]
                )


def _get_nc(repeats=1):
    if repeats not in _NC_CACHE:
        _NC_CACHE[repeats] = _build_program(repeats)
    return _NC_CACHE[repeats]


def _make_wmats(w):
    """Per-core lhsT weight mats, shape (128, 9, 128): wm[ic, t, oc]."""
    oc = np.arange(CPC)
    mats = []
    for cid in range(N_CORES):
        ws = np.asarray(w[cid * CPC : (cid + 1) * CPC], dtype=np.float32)
        wm = np.zeros((CPC, KH * KW, CPC), dtype=np.float32)
        for icg in range(2):
            ic = (oc // 2) * 2 + icg
            # advanced indexing on dims 0 and 2 -> result dims (pair, tap)
            wm[ic, :, oc] = ws[oc, icg].reshape(CPC, KH * KW)
        mats.append(wm)
    return mats


def _run(x, w, trace=False, **kwargs):
    nc = _get_nc()
    x = np.asarray(x, dtype=np.float32)
    wmats = _make_wmats(w)
    in_maps = [
        {
            "x": np.ascontiguousarray(x[:, cid * CPC : (cid + 1) * CPC]),
            "wm": wmats[cid],
        }
        for cid in range(N_CORES)
    ]
    res = run_bass_kernel_spmd(
        nc, in_maps, list(range(N_CORES)), trace=trace, **kwargs
    )
    y = np.concatenate(
        [res.results[i]["y"] for i in range(N_CORES)], axis=1
    )
    return y, res


def kernel(x, w):
    y, _ = _run(x, w, trace=False)
    return y



# revision 15
# speedup vs baseline: 2.0887x; 2.0887x over previous
"""Grouped Conv2d (512 groups, 2->2 ch/group, 3x3 VALID) on 8 trn2 NeuronCores.

Band-matrix formulation, bf16 end-to-end:
  - Shard the 512 groups across 8 cores (64 groups = 128 channels per core).
  - Host pre-permutes each core's input slab to x_perm[n, ic, dy, g, j]
    (bf16) so SBUF partitions hold (ic, dy) = input-channel x input-row and
    each partition's DMA descriptor is a 7168-byte contiguous run (full
    360 GB/s rate).
  - The 3 vertical taps (kh) are folded into a banded stationary matrix
    lhsT[(ic,dy), (oc,oy)] = w[oc, ic, dy-oy, kw]; only the 3 horizontal
    taps (kw) remain as separate accumulating matmuls, with the kw shift
    taken directly from the rhs free-dim offset:
        psum[(oc,oy), j] += sum_{ic,dy} W_kw[(ic,dy),(oc,oy)] x[ic,dy,j+kw]
    -> 3 matmuls x 54 columns per group instead of 9 x (rows*54): 2.5x
    fewer PE cycles than the tap-per-matmul formulation.
  - Output leaves in y_perm[n, (oc,oy), g, j] (bf16, full-rate DMA); the
    host permutes back to NCHW float32.
"""

import sys

import numpy as np

for _p in ("/opt/trn_rl_repo",):
    if _p not in sys.path:
        sys.path.insert(0, _p)

import ml_dtypes

import concourse.bacc as bacc
import concourse.bass as bass
import concourse.tile as tile
from concourse import mybir
from concourse.bass_utils import run_bass_kernel_spmd

N_CORES = 8
B, C, H, W = 16, 1024, 56, 56
KH = KW = 3
HO, WO = H - KH + 1, W - KW + 1  # 54, 54
CPC = C // N_CORES  # 128 channels per core
G = CPC // 2  # 64 groups per core
P_IN = 2 * H  # 112 partitions: (ic, dy)
P_OUT = 2 * HO  # 108 lanes: (oc, oy)
GPT = 8  # groups per PSUM tile (8*54 = 432 fp32 <= 512 = one bank)
N_GB = G // GPT  # 8 psum tiles per batch

BF16 = ml_dtypes.bfloat16

_NC_CACHE = {}


def _build_program(repeats=1):
    nc = bacc.Bacc(
        "TRN2", target_bir_lowering=False, debug=False, num_devices=N_CORES
    )
    f32 = mybir.dt.float32
    bf16 = mybir.dt.bfloat16

    x_d = nc.declare_dram_parameter("x", [B, P_IN, G, W], bf16, isOutput=False)
    wb_d = nc.declare_dram_parameter(
        "wb", [P_IN, G, KW, P_OUT], bf16, isOutput=False
    )
    y_d = nc.declare_dram_parameter(
        "y", [B, P_OUT, G, WO], bf16, isOutput=True
    )

    with tile.TileContext(nc) as tc:
        with (
            tc.tile_pool(name="wpool", bufs=1) as wpool,
            tc.tile_pool(name="xpool", bufs=4) as xpool,
            tc.tile_pool(name="opool", bufs=4) as opool,
            tc.tile_pool(name="psum", bufs=8, space="PSUM") as ppool,
        ):
            wt = wpool.tile([P_IN, G, KW, P_OUT], bf16)

            def body(first):
                _emit_batches(nc, tc, xpool, opool, ppool, x_d, y_d,
                              wb_d, wt, first)

            if repeats == 1:
                body(True)
            else:
                body(True)
                with tc.For_i(0, repeats - 1):
                    body(False)
    nc.compile()
    return nc


def _emit_batches(nc, tc, xpool, opool, ppool, x_d, y_d, wb_d, wt,
                  first):
    f32 = mybir.dt.float32

    # Batch-0 input first so batch 0 can start as soon as the weight
    # chunks land; SP's sequencer carries only x DMAs (y DMAs go out on
    # the Activation sequencer) so x prefetch never stalls behind
    # eviction semaphores.
    xts = {}

    def load_x(n):
        xts[n] = xpool.tile([P_IN, G, W], mybir.dt.bfloat16, name="xt")
        nc.sync.dma_start(out=xts[n][:], in_=x_d[n])

    if first:
        # Weight chunk 0 first so the PE warm-up can begin ~2.6us in;
        # batch 0/1 inputs interleave with the remaining chunks.
        WCHUNK = 8
        nc.sync.dma_start(out=wt[:, :WCHUNK], in_=wb_d[:, :WCHUNK])
        load_x(0)
        for gc in range(1, G // WCHUNK):
            lo, hi = gc * WCHUNK, (gc + 1) * WCHUNK
            nc.sync.dma_start(out=wt[:, lo:hi], in_=wb_d[:, lo:hi])
            if gc == 3:
                load_x(1)

        # Dummy matmuls ramp the PE clock gate to 2.4 GHz while the weight
        # and first-input DMAs drain. The scratch tile comes from the psum
        # pool rotation so all 8 banks stay available to the batch loop.
        scr = ppool.tile([P_OUT, GPT, WO], f32, name="pt")
        for _ in range(12):
            nc.tensor.matmul(
                scr[:, :KW * P_OUT // WO, :], lhsT=wt[:, 0, 0, :],
                rhs=wt[:, 0, :, :], start=True, stop=True,
            )
    else:
        load_x(0)
        load_x(1)

    for n in range(B):
        xc = xts.pop(n)
        ot = opool.tile([P_OUT, G, WO], mybir.dt.bfloat16, name="ot")
        for gb in range(N_GB):
            pt = ppool.tile([P_OUT, GPT, WO], f32, name="pt")
            for gl in range(GPT):
                g = gb * GPT + gl
                for kw in range(KW):
                    nc.tensor.matmul(
                        pt[:, gl, :],
                        lhsT=wt[:, g, kw, :],
                        rhs=xc[:, g, kw:kw + WO],
                        start=(kw == 0),
                        stop=(kw == KW - 1),
                    )
            dst = ot[:, gb * GPT:(gb + 1) * GPT, :]
            if gb % 2 == 0:
                nc.scalar.activation(
                    dst, pt[:], mybir.ActivationFunctionType.Copy
                )
            else:
                nc.vector.tensor_copy(dst, pt[:])
            if gb == N_GB // 2 - 1:
                nc.scalar.dma_start(
                    out=y_d[n, :, :G // 2, :], in_=ot[:, :G // 2, :]
                )
            elif n == B - 1 and gb > N_GB // 2 - 1:
                # Final batch: ship each group-block as soon as it is
                # evicted so the drain tail after the last matmul is short.
                lo, hi = gb * GPT, (gb + 1) * GPT
                nc.scalar.dma_start(
                    out=y_d[n, :, lo:hi, :], in_=ot[:, lo:hi, :]
                )
        if n != B - 1:
            nc.scalar.dma_start(
                out=y_d[n, :, G // 2:, :], in_=ot[:, G // 2:, :]
            )
        if n + 2 < B:
            load_x(n + 2)


def _get_nc(repeats=1):
    if repeats not in _NC_CACHE:
        _NC_CACHE[repeats] = _build_program(repeats)
    return _NC_CACHE[repeats]


def _make_bands(w):
    """Per-core banded lhsT weights, shape (112, 64, 3, 108) bf16.

    bands[ic*56 + oy + kh, g, kw, oc*54 + oy] = w[2g+oc, ic, kh, kw]
    """
    w = np.asarray(w, dtype=np.float32)
    wg = w.reshape(G * N_CORES, 2, 2, KH, KW)  # [g_all, oc, ic, kh, kw]
    oy = np.arange(HO)
    mats = []
    for cid in range(N_CORES):
        ws = wg[cid * G:(cid + 1) * G]  # [G, oc, ic, kh, kw]
        bands = np.zeros((P_IN, G, KW, P_OUT), dtype=np.float32)
        for ic in range(2):
            for oc in range(2):
                for kh in range(KH):
                    # rows (54, G, KW) <- per-group per-kw scalar
                    bands[ic * H + oy + kh, :, :, oc * HO + oy] = (
                        ws[:, oc, ic, kh, :][None, :, :]
                    )
        mats.append(bands.astype(BF16))
    return mats


def _permute_x(x):
    """Full x -> per-core list of x_perm[n, (ic,dy), g, j] bf16."""
    x = np.asarray(x)
    out = []
    for cid in range(N_CORES):
        xs = x[:, cid * CPC:(cid + 1) * CPC].astype(BF16)
        # [n, g, ic, dy, j] -> [n, ic, dy, g, j]
        xp = xs.reshape(B, G, 2, H, W).transpose(0, 2, 3, 1, 4)
        out.append(np.ascontiguousarray(xp.reshape(B, P_IN, G, W)))
    return out


def _unpermute_y(res):
    """Per-core y_perm[n, (oc,oy), g, j] bf16 -> full f32 NCHW."""
    parts = []
    for cid in range(N_CORES):
        yp = np.asarray(res[cid]["y"]).astype(np.float32)
        # [n, oc, oy, g, j] -> [n, g, oc, oy, j]
        yc = yp.reshape(B, 2, HO, G, WO).transpose(0, 3, 1, 2, 4)
        parts.append(yc.reshape(B, CPC, HO, WO))
    return np.concatenate(parts, axis=1)


def _run(x, w, trace=False, **kwargs):
    nc = _get_nc()
    xps = _permute_x(x)
    bands = _make_bands(w)
    in_maps = [
        {"x": xps[cid], "wb": bands[cid]} for cid in range(N_CORES)
    ]
    res = run_bass_kernel_spmd(
        nc, in_maps, list(range(N_CORES)), trace=trace, **kwargs
    )
    y = _unpermute_y(res.results)
    return y, res


def kernel(x, w):
    y, _ = _run(x, w, trace=False)
    return y


# revision 40
# speedup vs baseline: 2.2203x; 1.0630x over previous
"""Grouped Conv2d (512 groups, 2->2 ch/group, 3x3 VALID) on 8 trn2 NeuronCores.

Band-matrix formulation, bf16 end-to-end:
  - Shard the 512 groups across 8 cores (64 groups = 128 channels per core).
  - Host pre-permutes each core's input slab to x_perm[n, ic, dy, g, j]
    (bf16) so SBUF partitions hold (ic, dy) = input-channel x input-row and
    each partition's DMA descriptor is a 7168-byte contiguous run (full
    360 GB/s rate).
  - The 3 vertical taps (kh) are folded into a banded stationary matrix
    lhsT[(ic,dy), (oc,oy)] = w[oc, ic, dy-oy, kw]; only the 3 horizontal
    taps (kw) remain as separate accumulating matmuls, with the kw shift
    taken directly from the rhs free-dim offset:
        psum[(oc,oy), j] += sum_{ic,dy} W_kw[(ic,dy),(oc,oy)] x[ic,dy,j+kw]
    -> 3 matmuls x 54 columns per group instead of 9 x (rows*54): 2.5x
    fewer PE cycles than the tap-per-matmul formulation.
  - Output leaves in y_perm[n, (oc,oy), g, j] (bf16, full-rate DMA); the
    host permutes back to NCHW float32.
"""

import sys

import numpy as np

for _p in ("/opt/trn_rl_repo",):
    if _p not in sys.path:
        sys.path.insert(0, _p)

import ml_dtypes

import concourse.bacc as bacc
import concourse.bass as bass
import concourse.tile as tile
from concourse import mybir
from concourse.bass_utils import run_bass_kernel_spmd

N_CORES = 8
B, C, H, W = 16, 1024, 56, 56
KH = KW = 3
HO, WO = H - KH + 1, W - KW + 1  # 54, 54
CPC = C // N_CORES  # 128 channels per core
G = CPC // 2  # 64 groups per core
P_IN = 2 * H  # 112 partitions: (ic, dy)
P_OUT = 2 * HO  # 108 lanes: (oc, oy)
GPT = 8  # groups per PSUM tile (8*54 = 432 fp32 <= 512 = one bank)
N_GB = G // GPT  # 8 psum tiles per batch

BF16 = ml_dtypes.bfloat16

_NC_CACHE = {}


def _build_program(repeats=1):
    nc = bacc.Bacc(
        "TRN2", target_bir_lowering=False, debug=False, num_devices=N_CORES
    )
    f32 = mybir.dt.float32
    bf16 = mybir.dt.bfloat16

    x_d = nc.declare_dram_parameter("x", [B, P_IN, G, W], bf16, isOutput=False)
    wb_d = nc.declare_dram_parameter(
        "wb", [P_IN, G, KW, P_OUT], bf16, isOutput=False
    )
    y_d = nc.declare_dram_parameter(
        "y", [B, P_OUT, G, WO], bf16, isOutput=True
    )

    with tile.TileContext(nc) as tc:
        with (
            tc.tile_pool(name="wpool", bufs=1) as wpool,
            tc.tile_pool(name="xpool", bufs=5) as xpool,
            tc.tile_pool(name="opool", bufs=4) as opool,
            tc.tile_pool(name="psum", bufs=8, space="PSUM") as ppool,
        ):
            wt = wpool.tile([P_IN, G, KW, P_OUT], bf16)

            def body(first):
                _emit_batches(nc, tc, xpool, opool, ppool, x_d, y_d,
                              wb_d, wt, first)

            if repeats == 1:
                body(True)
            else:
                body(True)
                with tc.For_i(0, repeats - 1):
                    body(False)
    nc.compile()
    return nc


def _emit_batches(nc, tc, xpool, opool, ppool, x_d, y_d, wb_d, wt,
                  first):
    f32 = mybir.dt.float32

    # Batch-0 input first so batch 0 can start as soon as the weight
    # chunks land; SP's sequencer carries only x DMAs (y DMAs go out on
    # the Activation sequencer) so x prefetch never stalls behind
    # eviction semaphores.
    xts = {}

    def load_x(n):
        xts[n] = xpool.tile([P_IN, G, W], mybir.dt.bfloat16, name="xt")
        nc.sync.dma_start(out=xts[n][:], in_=x_d[n])

    PRE = 3  # batches interleaved by group-block during the weight load
    if first:
        # Weight chunk 0 first so the PE warm-up can begin ~2.6us in; the
        # first PRE inputs interleave with the early chunks so the
        # prologue jobs below are released roughly in emission order and
        # the 12.9us weight load hides behind compute.
        WCHUNK = 8
        for gc in range(G // WCHUNK):
            lo, hi = gc * WCHUNK, (gc + 1) * WCHUNK
            nc.sync.dma_start(out=wt[:, lo:hi], in_=wb_d[:, lo:hi])
            if gc < PRE:
                load_x(gc)
        load_x(PRE)

        # Dummy matmuls ramp the PE clock gate to 2.4 GHz while the weight
        # and first-input DMAs drain.
        for s in range(8):
            scr = ppool.tile([P_OUT, GPT, WO], f32, name="pt")
            for _ in range(2 if s < 4 else 1):
                nc.tensor.matmul(
                    scr[:, :KW * P_OUT // WO, :], lhsT=wt[:, 0, 0, :],
                    rhs=wt[:, 0, :, :], start=True, stop=True,
                )
    else:
        for k in range(PRE + 1):
            load_x(k)

    ots = {}

    def emit_block(n, gb):
        if gb == 0:
            ots[n] = opool.tile([P_OUT, G, WO], mybir.dt.bfloat16, name="ot")
        xc, ot = xts[n], ots[n]
        pt = ppool.tile([P_OUT, GPT, WO], f32, name="pt")
        for gl in range(GPT):
            g = gb * GPT + gl
            for kw in range(KW):
                nc.tensor.matmul(
                    pt[:, gl, :],
                    lhsT=wt[:, g, kw, :],
                    rhs=xc[:, g, kw:kw + WO],
                    start=(kw == 0),
                    stop=(kw == KW - 1),
                )
        # First-half blocks evict on DVE, second half on Activation; the
        # Activation-issued y DMAs then depend on their evictions via
        # same-engine program order only.
        dst = ot[:, gb * GPT:(gb + 1) * GPT, :]
        if n == B - 1 and gb == N_GB - 1:
            # Final block: evict half on DVE and half on Activation in
            # parallel, then one quarter DMA, so the drain after the last
            # matmul is as short as possible.
            half = GPT // 2
            nc.vector.tensor_copy(
                ot[:, gb * GPT:gb * GPT + half, :], pt[:, :half, :]
            )
            nc.scalar.activation(
                ot[:, gb * GPT + half:(gb + 1) * GPT, :], pt[:, half:, :],
                mybir.ActivationFunctionType.Copy,
            )
            nc.scalar.dma_start(
                out=y_d[n, :, 3 * G // 4:, :], in_=ot[:, 3 * G // 4:, :]
            )
        elif gb < N_GB // 2:
            nc.vector.tensor_copy(dst, pt[:])
        else:
            nc.scalar.activation(
                dst, pt[:], mybir.ActivationFunctionType.Copy
            )
        if gb == N_GB // 2 - 1:
            nc.sync.dma_start(
                out=y_d[n, :, :G // 2, :], in_=ot[:, :G // 2, :]
            )
        elif gb == N_GB - 1:
            if n == B - 1:
                # Remaining quarter (gb4-6's groups) on SP.
                nc.sync.dma_start(
                    out=y_d[n, :, G // 2:3 * G // 4, :],
                    in_=ot[:, G // 2:3 * G // 4, :]
                )
            else:
                nc.scalar.dma_start(
                    out=y_d[n, :, G // 2:, :], in_=ot[:, G // 2:, :]
                )
            xts.pop(n)
            ots.pop(n)
            nxt = n + PRE + 1
            if nxt < B:
                load_x(nxt)

    # Prologue: batches 0..PRE-1 interleaved by group-block in release
    # order (job (n, gb) is released when x(n) and weight chunk gb have
    # both landed), then the steady per-batch pipeline.
    PROLOGUE = [
        (0, 0), (0, 1), (1, 0), (1, 1), (0, 2), (1, 2),
        (2, 0), (2, 1), (2, 2),
    ] + [(n, gb) for gb in range(3, N_GB) for n in range(PRE)]
    for n, gb in PROLOGUE:
        emit_block(n, gb)
    for n in range(PRE, B):
        for gb in range(N_GB):
            emit_block(n, gb)


def _get_nc(repeats=1):
    if repeats not in _NC_CACHE:
        _NC_CACHE[repeats] = _build_program(repeats)
    return _NC_CACHE[repeats]


def _make_bands(w):
    """Per-core banded lhsT weights, shape (112, 64, 3, 108) bf16.

    bands[ic*56 + oy + kh, g, kw, oc*54 + oy] = w[2g+oc, ic, kh, kw]
    """
    w = np.asarray(w, dtype=np.float32)
    wg = w.reshape(G * N_CORES, 2, 2, KH, KW)  # [g_all, oc, ic, kh, kw]
    oy = np.arange(HO)
    mats = []
    for cid in range(N_CORES):
        ws = wg[cid * G:(cid + 1) * G]  # [G, oc, ic, kh, kw]
        bands = np.zeros((P_IN, G, KW, P_OUT), dtype=np.float32)
        for ic in range(2):
            for oc in range(2):
                for kh in range(KH):
                    # rows (54, G, KW) <- per-group per-kw scalar
                    bands[ic * H + oy + kh, :, :, oc * HO + oy] = (
                        ws[:, oc, ic, kh, :][None, :, :]
                    )
        mats.append(bands.astype(BF16))
    return mats


def _permute_x(x):
    """Full x -> per-core list of x_perm[n, (ic,dy), g, j] bf16."""
    x = np.asarray(x)
    out = []
    for cid in range(N_CORES):
        xs = x[:, cid * CPC:(cid + 1) * CPC].astype(BF16)
        # [n, g, ic, dy, j] -> [n, ic, dy, g, j]
        xp = xs.reshape(B, G, 2, H, W).transpose(0, 2, 3, 1, 4)
        out.append(np.ascontiguousarray(xp.reshape(B, P_IN, G, W)))
    return out


def _unpermute_y(res):
    """Per-core y_perm[n, (oc,oy), g, j] bf16 -> full f32 NCHW."""
    parts = []
    for cid in range(N_CORES):
        yp = np.asarray(res[cid]["y"]).astype(np.float32)
        # [n, oc, oy, g, j] -> [n, g, oc, oy, j]
        yc = yp.reshape(B, 2, HO, G, WO).transpose(0, 3, 1, 2, 4)
        parts.append(yc.reshape(B, CPC, HO, WO))
    return np.concatenate(parts, axis=1)


def _run(x, w, trace=False, **kwargs):
    nc = _get_nc()
    xps = _permute_x(x)
    bands = _make_bands(w)
    in_maps = [
        {"x": xps[cid], "wb": bands[cid]} for cid in range(N_CORES)
    ]
    res = run_bass_kernel_spmd(
        nc, in_maps, list(range(N_CORES)), trace=trace, **kwargs
    )
    y = _unpermute_y(res.results)
    return y, res


def kernel(x, w):
    y, _ = _run(x, w, trace=False)
    return y


# revision 52
# speedup vs baseline: 2.2339x; 1.0061x over previous
"""Grouped Conv2d (512 groups, 2->2 ch/group, 3x3 VALID) on 8 trn2 NeuronCores.

Band-matrix formulation, bf16 end-to-end:
  - Shard the 512 groups across 8 cores (64 groups = 128 channels per core).
  - Host pre-permutes each core's input slab to x_perm[n, ic, dy, g, j]
    (bf16) so SBUF partitions hold (ic, dy) = input-channel x input-row and
    each partition's DMA descriptor is a 7168-byte contiguous run (full
    360 GB/s rate).
  - The 3 vertical taps (kh) are folded into a banded stationary matrix
    lhsT[(ic,dy), (oc,oy)] = w[oc, ic, dy-oy, kw]; only the 3 horizontal
    taps (kw) remain as separate accumulating matmuls, with the kw shift
    taken directly from the rhs free-dim offset:
        psum[(oc,oy), j] += sum_{ic,dy} W_kw[(ic,dy),(oc,oy)] x[ic,dy,j+kw]
    -> 3 matmuls x 54 columns per group instead of 9 x (rows*54): 2.5x
    fewer PE cycles than the tap-per-matmul formulation.
  - Output leaves in y_perm[n, (oc,oy), g, j] (bf16, full-rate DMA); the
    host permutes back to NCHW float32.
"""

import sys

import numpy as np

for _p in ("/opt/trn_rl_repo",):
    if _p not in sys.path:
        sys.path.insert(0, _p)

import ml_dtypes

import concourse.bacc as bacc
import concourse.bass as bass
import concourse.tile as tile
from concourse import mybir
from concourse.bass_utils import run_bass_kernel_spmd

N_CORES = 8
B, C, H, W = 16, 1024, 56, 56
KH = KW = 3
HO, WO = H - KH + 1, W - KW + 1  # 54, 54
CPC = C // N_CORES  # 128 channels per core
G = CPC // 2  # 64 groups per core
P_IN = 2 * H  # 112 partitions: (ic, dy)
P_OUT = 2 * HO  # 108 lanes: (oc, oy)
GPT = 8  # groups per PSUM tile (8*54 = 432 fp32 <= 512 = one bank)
N_GB = G // GPT  # 8 psum tiles per batch

BF16 = ml_dtypes.bfloat16

_NC_CACHE = {}


def _build_program(repeats=1):
    nc = bacc.Bacc(
        "TRN2", target_bir_lowering=False, debug=False, num_devices=N_CORES
    )
    f32 = mybir.dt.float32
    bf16 = mybir.dt.bfloat16

    x_d = nc.declare_dram_parameter("x", [B, P_IN, G, W], bf16, isOutput=False)
    wb_d = nc.declare_dram_parameter(
        "wb", [P_IN, G, KW, P_OUT], bf16, isOutput=False
    )
    y_d = nc.declare_dram_parameter(
        "y", [B, P_OUT, G, WO], bf16, isOutput=True
    )

    with tile.TileContext(nc) as tc:
        with (
            tc.tile_pool(name="wpool", bufs=1) as wpool,
            tc.tile_pool(name="xpool", bufs=5) as xpool,
            tc.tile_pool(name="opool", bufs=4) as opool,
            tc.tile_pool(name="psum", bufs=8, space="PSUM") as ppool,
        ):
            wt = wpool.tile([P_IN, G, KW, P_OUT], bf16)

            def body(first):
                _emit_batches(nc, tc, xpool, opool, ppool, x_d, y_d,
                              wb_d, wt, first)

            if repeats == 1:
                body(True)
            else:
                body(True)
                with tc.For_i(0, repeats - 1):
                    body(False)
    nc.compile()
    return nc


def _emit_batches(nc, tc, xpool, opool, ppool, x_d, y_d, wb_d, wt,
                  first):
    f32 = mybir.dt.float32

    # Batch-0 input first so batch 0 can start as soon as the weight
    # chunks land; SP's sequencer carries only x DMAs (y DMAs go out on
    # the Activation sequencer) so x prefetch never stalls behind
    # eviction semaphores.
    xts = {}

    def load_x(n):
        xts[n] = xpool.tile([P_IN, G, W], mybir.dt.bfloat16, name="xt")
        nc.sync.dma_start(out=xts[n][:], in_=x_d[n])

    PRE = 3  # batches interleaved by group-block during the weight load
    if first:
        # Weight chunk 0 first so the PE warm-up can begin ~2.6us in; the
        # first PRE inputs interleave with the early chunks so the
        # prologue jobs below are released roughly in emission order and
        # the 12.9us weight load hides behind compute.
        WCHUNK = 8
        for gc in range(G // WCHUNK):
            lo, hi = gc * WCHUNK, (gc + 1) * WCHUNK
            nc.sync.dma_start(out=wt[:, lo:hi], in_=wb_d[:, lo:hi])
            if gc < PRE:
                load_x(gc)
        load_x(PRE)

        # Dummy matmuls ramp the PE clock gate to 2.4 GHz while the weight
        # and first-input DMAs drain.
        for s in range(8):
            scr = ppool.tile([P_OUT, GPT, WO], f32, name="pt")
            for _ in range(2 if s < 4 else 1):
                nc.tensor.matmul(
                    scr[:, :KW * P_OUT // WO, :], lhsT=wt[:, 0, 0, :],
                    rhs=wt[:, 0, :, :], start=True, stop=True,
                )
    else:
        for k in range(PRE + 1):
            load_x(k)

    ots = {}

    def emit_block(n, gb):
        if gb == 0:
            ots[n] = opool.tile([P_OUT, G, WO], mybir.dt.bfloat16, name="ot")
        xc, ot = xts[n], ots[n]
        pt = ppool.tile([P_OUT, GPT, WO], f32, name="pt")
        for gl in range(GPT):
            g = gb * GPT + gl
            for kw in range(KW):
                nc.tensor.matmul(
                    pt[:, gl, :],
                    lhsT=wt[:, g, kw, :],
                    rhs=xc[:, g, kw:kw + WO],
                    start=(kw == 0),
                    stop=(kw == KW - 1),
                )
        # First-half blocks evict on DVE, second half on Activation; the
        # Activation-issued y DMAs then depend on their evictions via
        # same-engine program order only.
        dst = ot[:, gb * GPT:(gb + 1) * GPT, :]
        if n == B - 1 and gb == N_GB - 1:
            # Final block: evict on DVE (whose sequencer is idle by now)
            # and ship the last quarter from the idle SP sequencer, so the
            # drain does not queue behind Activation's backlog.
            nc.vector.tensor_copy(dst, pt[:])
            nc.sync.dma_start(
                out=y_d[n, :, 3 * G // 4:, :], in_=ot[:, 3 * G // 4:, :]
            )
        elif gb < N_GB // 2:
            nc.vector.tensor_copy(dst, pt[:])
        else:
            nc.scalar.activation(
                dst, pt[:], mybir.ActivationFunctionType.Copy
            )
        if gb == N_GB // 2 - 1:
            nc.sync.dma_start(
                out=y_d[n, :, :G // 2, :], in_=ot[:, :G // 2, :]
            )
        elif gb == N_GB - 1:
            if n == B - 1:
                # Remaining quarter (gb4-6's groups) on SP.
                nc.sync.dma_start(
                    out=y_d[n, :, G // 2:3 * G // 4, :],
                    in_=ot[:, G // 2:3 * G // 4, :]
                )
            else:
                nc.scalar.dma_start(
                    out=y_d[n, :, G // 2:, :], in_=ot[:, G // 2:, :]
                )
            xts.pop(n)
            ots.pop(n)
            nxt = n + PRE + 1
            if nxt < B:
                load_x(nxt)

    # Prologue: batches 0..PRE-1 interleaved by group-block in release
    # order (job (n, gb) is released when x(n) and weight chunk gb have
    # both landed), then the steady per-batch pipeline.
    PROLOGUE = [
        (0, 0), (0, 1), (1, 0), (1, 1), (0, 2), (1, 2),
        (2, 0), (2, 1), (2, 2),
    ] + [(n, gb) for gb in range(3, N_GB) for n in range(PRE)]
    for n, gb in PROLOGUE:
        emit_block(n, gb)
    for n in range(PRE, B):
        for gb in range(N_GB):
            emit_block(n, gb)


def _get_nc(repeats=1):
    if repeats not in _NC_CACHE:
        _NC_CACHE[repeats] = _build_program(repeats)
    return _NC_CACHE[repeats]


def _make_bands(w):
    """Per-core banded lhsT weights, shape (112, 64, 3, 108) bf16.

    bands[ic*56 + oy + kh, g, kw, oc*54 + oy] = w[2g+oc, ic, kh, kw]
    """
    w = np.asarray(w, dtype=np.float32)
    wg = w.reshape(G * N_CORES, 2, 2, KH, KW)  # [g_all, oc, ic, kh, kw]
    oy = np.arange(HO)
    mats = []
    for cid in range(N_CORES):
        ws = wg[cid * G:(cid + 1) * G]  # [G, oc, ic, kh, kw]
        bands = np.zeros((P_IN, G, KW, P_OUT), dtype=np.float32)
        for ic in range(2):
            for oc in range(2):
                for kh in range(KH):
                    # rows (54, G, KW) <- per-group per-kw scalar
                    bands[ic * H + oy + kh, :, :, oc * HO + oy] = (
                        ws[:, oc, ic, kh, :][None, :, :]
                    )
        mats.append(bands.astype(BF16))
    return mats


def _permute_x(x):
    """Full x -> per-core list of x_perm[n, (ic,dy), g, j] bf16."""
    x = np.asarray(x)
    out = []
    for cid in range(N_CORES):
        xs = x[:, cid * CPC:(cid + 1) * CPC].astype(BF16)
        # [n, g, ic, dy, j] -> [n, ic, dy, g, j]
        xp = xs.reshape(B, G, 2, H, W).transpose(0, 2, 3, 1, 4)
        out.append(np.ascontiguousarray(xp.reshape(B, P_IN, G, W)))
    return out


def _unpermute_y(res):
    """Per-core y_perm[n, (oc,oy), g, j] bf16 -> full f32 NCHW."""
    parts = []
    for cid in range(N_CORES):
        yp = np.asarray(res[cid]["y"]).astype(np.float32)
        # [n, oc, oy, g, j] -> [n, g, oc, oy, j]
        yc = yp.reshape(B, 2, HO, G, WO).transpose(0, 3, 1, 2, 4)
        parts.append(yc.reshape(B, CPC, HO, WO))
    return np.concatenate(parts, axis=1)


def _run(x, w, trace=False, **kwargs):
    nc = _get_nc()
    xps = _permute_x(x)
    bands = _make_bands(w)
    in_maps = [
        {"x": xps[cid], "wb": bands[cid]} for cid in range(N_CORES)
    ]
    res = run_bass_kernel_spmd(
        nc, in_maps, list(range(N_CORES)), trace=trace, **kwargs
    )
    y = _unpermute_y(res.results)
    return y, res


def kernel(x, w):
    y, _ = _run(x, w, trace=False)
    return y


# revision 56
# speedup vs baseline: 2.2341x; 1.0001x over previous
"""Grouped Conv2d (512 groups, 2->2 ch/group, 3x3 VALID) on 8 trn2 NeuronCores.

Band-matrix formulation, bf16 end-to-end:
  - Shard the 512 groups across 8 cores (64 groups = 128 channels per core).
  - Host pre-permutes each core's input slab to x_perm[n, ic, dy, g, j]
    (bf16) so SBUF partitions hold (ic, dy) = input-channel x input-row and
    each partition's DMA descriptor is a 7168-byte contiguous run (full
    360 GB/s rate).
  - The 3 vertical taps (kh) are folded into a banded stationary matrix
    lhsT[(ic,dy), (oc,oy)] = w[oc, ic, dy-oy, kw]; only the 3 horizontal
    taps (kw) remain as separate accumulating matmuls, with the kw shift
    taken directly from the rhs free-dim offset:
        psum[(oc,oy), j] += sum_{ic,dy} W_kw[(ic,dy),(oc,oy)] x[ic,dy,j+kw]
    -> 3 matmuls x 54 columns per group instead of 9 x (rows*54): 2.5x
    fewer PE cycles than the tap-per-matmul formulation.
  - Output leaves in y_perm[n, (oc,oy), g, j] (bf16, full-rate DMA); the
    host permutes back to NCHW float32.
"""

import sys

import numpy as np

for _p in ("/opt/trn_rl_repo",):
    if _p not in sys.path:
        sys.path.insert(0, _p)

import ml_dtypes

import concourse.bacc as bacc
import concourse.bass as bass
import concourse.tile as tile
from concourse import mybir
from concourse.bass_utils import run_bass_kernel_spmd

N_CORES = 8
B, C, H, W = 16, 1024, 56, 56
KH = KW = 3
HO, WO = H - KH + 1, W - KW + 1  # 54, 54
CPC = C // N_CORES  # 128 channels per core
G = CPC // 2  # 64 groups per core
P_IN = 2 * H  # 112 partitions: (ic, dy)
P_OUT = 2 * HO  # 108 lanes: (oc, oy)
GPT = 8  # groups per PSUM tile (8*54 = 432 fp32 <= 512 = one bank)
N_GB = G // GPT  # 8 psum tiles per batch

BF16 = ml_dtypes.bfloat16

_NC_CACHE = {}


def _build_program(repeats=1):
    nc = bacc.Bacc(
        "TRN2", target_bir_lowering=False, debug=False, num_devices=N_CORES
    )
    f32 = mybir.dt.float32
    bf16 = mybir.dt.bfloat16

    x_d = nc.declare_dram_parameter("x", [B, P_IN, G, W], bf16, isOutput=False)
    wb_d = nc.declare_dram_parameter(
        "wb", [P_IN, G, KW, P_OUT], bf16, isOutput=False
    )
    y_d = nc.declare_dram_parameter(
        "y", [B, P_OUT, G, WO], bf16, isOutput=True
    )

    with tile.TileContext(nc) as tc:
        with (
            tc.tile_pool(name="wpool", bufs=1) as wpool,
            tc.tile_pool(name="xpool", bufs=5) as xpool,
            tc.tile_pool(name="opool", bufs=4) as opool,
            tc.tile_pool(name="psum", bufs=8, space="PSUM") as ppool,
        ):
            wt = wpool.tile([P_IN, G, KW, P_OUT], bf16)

            def body(first):
                _emit_batches(nc, tc, xpool, opool, ppool, x_d, y_d,
                              wb_d, wt, first)

            if repeats == 1:
                body(True)
            else:
                body(True)
                with tc.For_i(0, repeats - 1):
                    body(False)
    nc.compile()
    return nc


def _emit_batches(nc, tc, xpool, opool, ppool, x_d, y_d, wb_d, wt,
                  first):
    f32 = mybir.dt.float32

    # Batch-0 input first so batch 0 can start as soon as the weight
    # chunks land; SP's sequencer carries only x DMAs (y DMAs go out on
    # the Activation sequencer) so x prefetch never stalls behind
    # eviction semaphores.
    xts = {}

    def load_x(n):
        xts[n] = xpool.tile([P_IN, G, W], mybir.dt.bfloat16, name="xt")
        nc.sync.dma_start(out=xts[n][:], in_=x_d[n])

    PRE = 3  # batches interleaved by group-block during the weight load
    if first:
        # Weight chunk 0 first so the PE warm-up can begin ~2.6us in; the
        # first PRE inputs interleave with the early chunks so the
        # prologue jobs below are released roughly in emission order and
        # the 12.9us weight load hides behind compute.
        WCHUNK = 4
        for gc in range(G // WCHUNK):
            lo, hi = gc * WCHUNK, (gc + 1) * WCHUNK
            nc.sync.dma_start(out=wt[:, lo:hi], in_=wb_d[:, lo:hi])
            if gc in (0, 2, 4):
                load_x(gc // 2)
        load_x(PRE)

        # Dummy matmuls ramp the PE clock gate to 2.4 GHz while the weight
        # and first-input DMAs drain.
        for s in range(8):
            scr = ppool.tile([P_OUT, GPT, WO], f32, name="pt")
            for _ in range(2 if s < 4 else 1):
                nc.tensor.matmul(
                    scr[:, :KW * P_OUT // WO, :], lhsT=wt[:, 0, 0, :],
                    rhs=wt[:, 0, :, :], start=True, stop=True,
                )
    else:
        for k in range(PRE + 1):
            load_x(k)

    ots = {}

    def emit_block(n, gb):
        if gb == 0:
            ots[n] = opool.tile([P_OUT, G, WO], mybir.dt.bfloat16, name="ot")
        xc, ot = xts[n], ots[n]
        pt = ppool.tile([P_OUT, GPT, WO], f32, name="pt")
        for gl in range(GPT):
            g = gb * GPT + gl
            for kw in range(KW):
                nc.tensor.matmul(
                    pt[:, gl, :],
                    lhsT=wt[:, g, kw, :],
                    rhs=xc[:, g, kw:kw + WO],
                    start=(kw == 0),
                    stop=(kw == KW - 1),
                )
        # First-half blocks evict on DVE, second half on Activation; the
        # Activation-issued y DMAs then depend on their evictions via
        # same-engine program order only.
        dst = ot[:, gb * GPT:(gb + 1) * GPT, :]
        if n == B - 1 and gb == N_GB - 1:
            # Final block: evict on DVE (whose sequencer is idle by now)
            # and ship the last quarter from the idle SP sequencer, so the
            # drain does not queue behind Activation's backlog.
            nc.vector.tensor_copy(dst, pt[:])
            nc.sync.dma_start(
                out=y_d[n, :, 3 * G // 4:, :], in_=ot[:, 3 * G // 4:, :]
            )
        elif gb < N_GB // 2:
            nc.vector.tensor_copy(dst, pt[:])
        else:
            nc.scalar.activation(
                dst, pt[:], mybir.ActivationFunctionType.Copy
            )
        if gb == N_GB // 2 - 1:
            nc.sync.dma_start(
                out=y_d[n, :, :G // 2, :], in_=ot[:, :G // 2, :]
            )
        elif gb == N_GB - 1:
            if n == B - 1:
                # Remaining quarter (gb4-6's groups) on SP.
                nc.sync.dma_start(
                    out=y_d[n, :, G // 2:3 * G // 4, :],
                    in_=ot[:, G // 2:3 * G // 4, :]
                )
            else:
                nc.scalar.dma_start(
                    out=y_d[n, :, G // 2:, :], in_=ot[:, G // 2:, :]
                )
            xts.pop(n)
            ots.pop(n)
            nxt = n + PRE + 1
            if nxt < B:
                load_x(nxt)

    # Prologue: batches 0..PRE-1 interleaved by group-block in release
    # order (job (n, gb) is released when x(n) and weight chunk gb have
    # both landed), then the steady per-batch pipeline.
    PROLOGUE = [
        (0, 0), (0, 1), (1, 0), (1, 1), (0, 2), (1, 2),
        (2, 0), (2, 1), (2, 2),
    ] + [(n, gb) for gb in range(3, N_GB) for n in range(PRE)]
    for n, gb in PROLOGUE:
        emit_block(n, gb)
    for n in range(PRE, B):
        for gb in range(N_GB):
            emit_block(n, gb)


def _get_nc(repeats=1):
    if repeats not in _NC_CACHE:
        _NC_CACHE[repeats] = _build_program(repeats)
    return _NC_CACHE[repeats]


def _make_bands(w):
    """Per-core banded lhsT weights, shape (112, 64, 3, 108) bf16.

    bands[ic*56 + oy + kh, g, kw, oc*54 + oy] = w[2g+oc, ic, kh, kw]
    """
    w = np.asarray(w, dtype=np.float32)
    wg = w.reshape(G * N_CORES, 2, 2, KH, KW)  # [g_all, oc, ic, kh, kw]
    oy = np.arange(HO)
    mats = []
    for cid in range(N_CORES):
        ws = wg[cid * G:(cid + 1) * G]  # [G, oc, ic, kh, kw]
        bands = np.zeros((P_IN, G, KW, P_OUT), dtype=np.float32)
        for ic in range(2):
            for oc in range(2):
                for kh in range(KH):
                    # rows (54, G, KW) <- per-group per-kw scalar
                    bands[ic * H + oy + kh, :, :, oc * HO + oy] = (
                        ws[:, oc, ic, kh, :][None, :, :]
                    )
        mats.append(bands.astype(BF16))
    return mats


def _permute_x(x):
    """Full x -> per-core list of x_perm[n, (ic,dy), g, j] bf16."""
    x = np.asarray(x)
    out = []
    for cid in range(N_CORES):
        xs = x[:, cid * CPC:(cid + 1) * CPC].astype(BF16)
        # [n, g, ic, dy, j] -> [n, ic, dy, g, j]
        xp = xs.reshape(B, G, 2, H, W).transpose(0, 2, 3, 1, 4)
        out.append(np.ascontiguousarray(xp.reshape(B, P_IN, G, W)))
    return out


def _unpermute_y(res):
    """Per-core y_perm[n, (oc,oy), g, j] bf16 -> full f32 NCHW."""
    parts = []
    for cid in range(N_CORES):
        yp = np.asarray(res[cid]["y"]).astype(np.float32)
        # [n, oc, oy, g, j] -> [n, g, oc, oy, j]
        yc = yp.reshape(B, 2, HO, G, WO).transpose(0, 3, 1, 2, 4)
        parts.append(yc.reshape(B, CPC, HO, WO))
    return np.concatenate(parts, axis=1)


def _run(x, w, trace=False, **kwargs):
    nc = _get_nc()
    xps = _permute_x(x)
    bands = _make_bands(w)
    in_maps = [
        {"x": xps[cid], "wb": bands[cid]} for cid in range(N_CORES)
    ]
    res = run_bass_kernel_spmd(
        nc, in_maps, list(range(N_CORES)), trace=trace, **kwargs
    )
    y = _unpermute_y(res.results)
    return y, res


def kernel(x, w):
    y, _ = _run(x, w, trace=False)
    return y


# revision 59
# speedup vs baseline: 2.2354x; 1.0006x over previous
"""Grouped Conv2d (512 groups, 2->2 ch/group, 3x3 VALID) on 8 trn2 NeuronCores.

Band-matrix formulation, bf16 end-to-end:
  - Shard the 512 groups across 8 cores (64 groups = 128 channels per core).
  - Host pre-permutes each core's input slab to x_perm[n, ic, dy, g, j]
    (bf16) so SBUF partitions hold (ic, dy) = input-channel x input-row and
    each partition's DMA descriptor is a 7168-byte contiguous run (full
    360 GB/s rate).
  - The 3 vertical taps (kh) are folded into a banded stationary matrix
    lhsT[(ic,dy), (oc,oy)] = w[oc, ic, dy-oy, kw]; only the 3 horizontal
    taps (kw) remain as separate accumulating matmuls, with the kw shift
    taken directly from the rhs free-dim offset:
        psum[(oc,oy), j] += sum_{ic,dy} W_kw[(ic,dy),(oc,oy)] x[ic,dy,j+kw]
    -> 3 matmuls x 54 columns per group instead of 9 x (rows*54): 2.5x
    fewer PE cycles than the tap-per-matmul formulation.
  - Output leaves in y_perm[n, (oc,oy), g, j] (bf16, full-rate DMA); the
    host permutes back to NCHW float32.
"""

import sys

import numpy as np

for _p in ("/opt/trn_rl_repo",):
    if _p not in sys.path:
        sys.path.insert(0, _p)

import ml_dtypes

import concourse.bacc as bacc
import concourse.bass as bass
import concourse.tile as tile
from concourse import mybir
from concourse.bass_utils import run_bass_kernel_spmd

N_CORES = 8
B, C, H, W = 16, 1024, 56, 56
KH = KW = 3
HO, WO = H - KH + 1, W - KW + 1  # 54, 54
CPC = C // N_CORES  # 128 channels per core
G = CPC // 2  # 64 groups per core
P_IN = 2 * H  # 112 partitions: (ic, dy)
P_OUT = 2 * HO  # 108 lanes: (oc, oy)
GPT = 8  # groups per PSUM tile (8*54 = 432 fp32 <= 512 = one bank)
N_GB = G // GPT  # 8 psum tiles per batch

BF16 = ml_dtypes.bfloat16

_NC_CACHE = {}


def _build_program(repeats=1):
    nc = bacc.Bacc(
        "TRN2", target_bir_lowering=False, debug=False, num_devices=N_CORES
    )
    f32 = mybir.dt.float32
    bf16 = mybir.dt.bfloat16

    x_d = nc.declare_dram_parameter("x", [B, P_IN, G, W], bf16, isOutput=False)
    wb_d = nc.declare_dram_parameter(
        "wb", [P_IN, G, KW, P_OUT], bf16, isOutput=False
    )
    y_d = nc.declare_dram_parameter(
        "y", [B, P_OUT, G, WO], bf16, isOutput=True
    )

    with tile.TileContext(nc) as tc:
        with (
            tc.tile_pool(name="wpool", bufs=1) as wpool,
            tc.tile_pool(name="xpool", bufs=5) as xpool,
            tc.tile_pool(name="opool", bufs=4) as opool,
            tc.tile_pool(name="psum", bufs=8, space="PSUM") as ppool,
        ):
            wt = wpool.tile([P_IN, G, KW, P_OUT], bf16)

            def body(first):
                _emit_batches(nc, tc, xpool, opool, ppool, x_d, y_d,
                              wb_d, wt, first)

            if repeats == 1:
                body(True)
            else:
                body(True)
                with tc.For_i(0, repeats - 1):
                    body(False)
    nc.compile()
    return nc


def _emit_batches(nc, tc, xpool, opool, ppool, x_d, y_d, wb_d, wt,
                  first):
    f32 = mybir.dt.float32

    # Batch-0 input first so batch 0 can start as soon as the weight
    # chunks land; SP's sequencer carries only x DMAs (y DMAs go out on
    # the Activation sequencer) so x prefetch never stalls behind
    # eviction semaphores.
    xts = {}

    def load_x(n):
        xts[n] = xpool.tile([P_IN, G, W], mybir.dt.bfloat16, name="xt")
        nc.sync.dma_start(out=xts[n][:], in_=x_d[n])

    PRE = 3  # batches interleaved by group-block during the weight load
    if first:
        # Weight chunk 0 first so the PE warm-up can begin ~2.6us in; the
        # first PRE inputs interleave with the early chunks so the
        # prologue jobs below are released roughly in emission order and
        # the 12.9us weight load hides behind compute.
        WCHUNK = 4
        for gc in range(G // WCHUNK):
            lo, hi = gc * WCHUNK, (gc + 1) * WCHUNK
            nc.sync.dma_start(out=wt[:, lo:hi], in_=wb_d[:, lo:hi])
            if gc in (0, 2, 4):
                load_x(gc // 2)
        load_x(PRE)

        # Dummy matmuls ramp the PE clock gate to 2.4 GHz while the weight
        # and first-input DMAs drain.
        for s in range(8):
            scr = ppool.tile([P_OUT, GPT, WO], f32, name="pt")
            for _ in range(2 if s < 4 else 1):
                nc.tensor.matmul(
                    scr[:, :KW * P_OUT // WO, :], lhsT=wt[:, 0, 0, :],
                    rhs=wt[:, 0, :, :], start=True, stop=True,
                )
    else:
        for k in range(PRE + 1):
            load_x(k)

    ots = {}

    def emit_block(n, gb):
        if gb == 0:
            ots[n] = opool.tile([P_OUT, G, WO], mybir.dt.bfloat16, name="ot")
        xc, ot = xts[n], ots[n]
        pt = ppool.tile([P_OUT, GPT, WO], f32, name="pt")
        for gl in range(GPT):
            g = gb * GPT + gl
            for kw in range(KW):
                nc.tensor.matmul(
                    pt[:, gl, :],
                    lhsT=wt[:, g, kw, :],
                    rhs=xc[:, g, kw:kw + WO],
                    start=(kw == 0),
                    stop=(kw == KW - 1),
                )
        # First-half blocks evict on DVE, second half on Activation; the
        # Activation-issued y DMAs then depend on their evictions via
        # same-engine program order only.
        dst = ot[:, gb * GPT:(gb + 1) * GPT, :]
        if n == B - 1 and gb == N_GB - 1:
            # Final block: evict on DVE (whose sequencer is idle by now)
            # and ship the last quarter from the idle SP sequencer, so the
            # drain does not queue behind Activation's backlog.
            nc.vector.tensor_copy(dst, pt[:])
            nc.sync.dma_start(
                out=y_d[n, :, G - GPT:, :], in_=ot[:, G - GPT:, :]
            )
        elif gb < N_GB // 2:
            nc.vector.tensor_copy(dst, pt[:])
        else:
            nc.scalar.activation(
                dst, pt[:], mybir.ActivationFunctionType.Copy
            )
        if gb == N_GB // 2 - 1:
            nc.sync.dma_start(
                out=y_d[n, :, :G // 2, :], in_=ot[:, :G // 2, :]
            )
        elif gb == N_GB - 1:
            if n == B - 1:
                # gb4-6's groups on SP (final block's 8 ship separately).
                nc.sync.dma_start(
                    out=y_d[n, :, G // 2:G - GPT, :],
                    in_=ot[:, G // 2:G - GPT, :]
                )
            else:
                nc.scalar.dma_start(
                    out=y_d[n, :, G // 2:, :], in_=ot[:, G // 2:, :]
                )
            xts.pop(n)
            ots.pop(n)
            nxt = n + PRE + 1
            if nxt < B:
                load_x(nxt)

    # Prologue: batches 0..PRE-1 interleaved by group-block in release
    # order (job (n, gb) is released when x(n) and weight chunk gb have
    # both landed), then the steady per-batch pipeline.
    PROLOGUE = [
        (0, 0), (0, 1), (1, 0), (1, 1), (0, 2), (1, 2),
        (2, 0), (2, 1), (2, 2),
    ] + [(n, gb) for gb in range(3, N_GB) for n in range(PRE)]
    for n, gb in PROLOGUE:
        emit_block(n, gb)
    for n in range(PRE, B):
        for gb in range(N_GB):
            emit_block(n, gb)


def _get_nc(repeats=1):
    if repeats not in _NC_CACHE:
        _NC_CACHE[repeats] = _build_program(repeats)
    return _NC_CACHE[repeats]


def _make_bands(w):
    """Per-core banded lhsT weights, shape (112, 64, 3, 108) bf16.

    bands[ic*56 + oy + kh, g, kw, oc*54 + oy] = w[2g+oc, ic, kh, kw]
    """
    w = np.asarray(w, dtype=np.float32)
    wg = w.reshape(G * N_CORES, 2, 2, KH, KW)  # [g_all, oc, ic, kh, kw]
    oy = np.arange(HO)
    mats = []
    for cid in range(N_CORES):
        ws = wg[cid * G:(cid + 1) * G]  # [G, oc, ic, kh, kw]
        bands = np.zeros((P_IN, G, KW, P_OUT), dtype=np.float32)
        for ic in range(2):
            for oc in range(2):
                for kh in range(KH):
                    # rows (54, G, KW) <- per-group per-kw scalar
                    bands[ic * H + oy + kh, :, :, oc * HO + oy] = (
                        ws[:, oc, ic, kh, :][None, :, :]
                    )
        mats.append(bands.astype(BF16))
    return mats


def _permute_x(x):
    """Full x -> per-core list of x_perm[n, (ic,dy), g, j] bf16."""
    x = np.asarray(x)
    out = []
    for cid in range(N_CORES):
        xs = x[:, cid * CPC:(cid + 1) * CPC].astype(BF16)
        # [n, g, ic, dy, j] -> [n, ic, dy, g, j]
        xp = xs.reshape(B, G, 2, H, W).transpose(0, 2, 3, 1, 4)
        out.append(np.ascontiguousarray(xp.reshape(B, P_IN, G, W)))
    return out


def _unpermute_y(res):
    """Per-core y_perm[n, (oc,oy), g, j] bf16 -> full f32 NCHW."""
    parts = []
    for cid in range(N_CORES):
        yp = np.asarray(res[cid]["y"]).astype(np.float32)
        # [n, oc, oy, g, j] -> [n, g, oc, oy, j]
        yc = yp.reshape(B, 2, HO, G, WO).transpose(0, 3, 1, 2, 4)
        parts.append(yc.reshape(B, CPC, HO, WO))
    return np.concatenate(parts, axis=1)


def _run(x, w, trace=False, **kwargs):
    nc = _get_nc()
    xps = _permute_x(x)
    bands = _make_bands(w)
    in_maps = [
        {"x": xps[cid], "wb": bands[cid]} for cid in range(N_CORES)
    ]
    res = run_bass_kernel_spmd(
        nc, in_maps, list(range(N_CORES)), trace=trace, **kwargs
    )
    y = _unpermute_y(res.results)
    return y, res


def kernel(x, w):
    y, _ = _run(x, w, trace=False)
    return y


# revision 65
# speedup vs baseline: 2.2463x; 1.0049x over previous
"""Grouped Conv2d (512 groups, 2->2 ch/group, 3x3 VALID) on 8 trn2 NeuronCores.

Band-matrix formulation, bf16 end-to-end; batches 0-3 run singly (their
DMAs interleave with the banded-weight load), batches 4-15 run as fused
pairs with two batches in the matmul moving dim (108 columns), which
halves the matmul count and avoids the per-matmul ceil-to-ns rounding.
"""

import sys

import numpy as np

for _p in ("/opt/trn_rl_repo",):
    if _p not in sys.path:
        sys.path.insert(0, _p)

import ml_dtypes

import concourse.bacc as bacc
import concourse.bass as bass
import concourse.tile as tile
from concourse import mybir
from concourse.bass_utils import run_bass_kernel_spmd

N_CORES = 8
B, C, H, W = 16, 1024, 56, 56
BH = 4              # head batches, processed singly
NP = (B - BH) // 2  # 6 fused pairs
KH = KW = 3
HO, WO = H - KH + 1, W - KW + 1  # 54, 54
CPC = C // N_CORES  # 128 channels per core
G = CPC // 2  # 64 groups per core
P_IN = 2 * H  # 112 partitions: (ic, dy)
P_OUT = 2 * HO  # 108 lanes: (oc, oy)
GPT_S = 8  # groups per psum tile, single-batch blocks (8*54 = 432 fp32)
GPT_P = 4  # groups per psum tile, pair blocks (4*2*54 = 432 fp32)
NGB_S = G // GPT_S  # 8 blocks per head batch
NGB_P = G // GPT_P  # 16 blocks per pair

BF16 = ml_dtypes.bfloat16

_NC_CACHE = {}


def _build_program(repeats=1):
    nc = bacc.Bacc(
        "TRN2", target_bir_lowering=False, debug=False, num_devices=N_CORES
    )
    f32 = mybir.dt.float32
    bf16 = mybir.dt.bfloat16

    x_d = nc.declare_dram_parameter(
        "x", [BH, P_IN, G, W], bf16, isOutput=False
    )
    xp_d = nc.declare_dram_parameter(
        "xp", [NP, P_IN, G, 2, W], bf16, isOutput=False
    )
    wb_d = nc.declare_dram_parameter(
        "wb", [P_IN, G, KW, P_OUT], bf16, isOutput=False
    )
    y_d = nc.declare_dram_parameter(
        "y", [BH, P_OUT, G, WO], bf16, isOutput=True
    )
    yp_d = nc.declare_dram_parameter(
        "yp", [NP, P_OUT, G, 2, WO], bf16, isOutput=True
    )

    with tile.TileContext(nc) as tc:
        with (
            tc.tile_pool(name="wpool", bufs=1) as wpool,
            tc.tile_pool(name="xspool", bufs=4) as xspool,
            tc.tile_pool(name="xppool", bufs=3) as xppool,
            tc.tile_pool(name="ospool", bufs=4) as ospool,
            tc.tile_pool(name="oppool", bufs=3) as oppool,
            tc.tile_pool(name="psum", bufs=8, space="PSUM") as ppool,
        ):
            wt = wpool.tile([P_IN, G, KW, P_OUT], bf16)

            def body(first):
                _emit(nc, tc, xspool, xppool, ospool, oppool, ppool,
                      x_d, xp_d, y_d, yp_d, wb_d, wt, first)

            if repeats == 1:
                body(True)
            else:
                body(True)
                with tc.For_i(0, repeats - 1):
                    body(False)
    nc.compile()
    return nc


def _emit(nc, tc, xspool, xppool, ospool, oppool, ppool,
          x_d, xp_d, y_d, yp_d, wb_d, wt, first):
    f32 = mybir.dt.float32
    bf16 = mybir.dt.bfloat16

    xts, xpts = {}, {}

    def load_x(n):
        xts[n] = xspool.tile([P_IN, G, W], bf16, name="xt")
        nc.sync.dma_start(out=xts[n][:], in_=x_d[n])

    def load_xp(p):
        xpts[p] = xppool.tile([P_IN, G, 2, W], bf16, name="xtp")
        nc.sync.dma_start(out=xpts[p][:], in_=xp_d[p])

    if first:
        # Weight chunks pace the prologue; the head-batch inputs
        # interleave with the early chunks so the 12.9us weight load
        # hides behind batches 0-2's compute.
        WCHUNK = 4
        for gc in range(G // WCHUNK):
            lo, hi = gc * WCHUNK, (gc + 1) * WCHUNK
            nc.sync.dma_start(out=wt[:, lo:hi], in_=wb_d[:, lo:hi])
            if gc in (0, 2, 4):
                load_x(gc // 2)
        load_x(3)

        # Dummy matmuls ramp the PE clock gate while the weight and
        # first-input DMAs drain.
        for s in range(8):
            scr = ppool.tile([P_OUT, 432], f32, name="pt")
            for _ in range(2 if s < 4 else 1):
                nc.tensor.matmul(
                    scr[:, :KW * P_OUT], lhsT=wt[:, 0, 0, :],
                    rhs=wt[:, 0, :, :], start=True, stop=True,
                )
    else:
        for k in range(BH):
            load_x(k)

    ots, otps = {}, {}

    def emit_head(n, gb):
        """One 8-group block of a single head batch."""
        if gb == 0:
            ots[n] = ospool.tile([P_OUT, G, WO], bf16, name="ot")
        xc, ot = xts[n], ots[n]
        pt = ppool.tile([P_OUT, 432], f32, name="pt")
        for gl in range(GPT_S):
            g = gb * GPT_S + gl
            for kw in range(KW):
                nc.tensor.matmul(
                    pt[:, gl * WO:(gl + 1) * WO],
                    lhsT=wt[:, g, kw, :],
                    rhs=xc[:, g, kw:kw + WO],
                    start=(kw == 0),
                    stop=(kw == KW - 1),
                )
        dst = ot[:, gb * GPT_S:(gb + 1) * GPT_S, :]
        if gb < NGB_S // 2:
            nc.vector.tensor_copy(dst, pt[:])
        else:
            nc.scalar.activation(
                dst, pt[:], mybir.ActivationFunctionType.Copy
            )
        if gb == NGB_S // 2 - 1:
            nc.sync.dma_start(
                out=y_d[n, :, :G // 2, :], in_=ot[:, :G // 2, :]
            )
        elif gb == NGB_S - 1:
            nc.scalar.dma_start(
                out=y_d[n, :, G // 2:, :], in_=ot[:, G // 2:, :]
            )
            xts.pop(n)
            ots.pop(n)

    def emit_pair(p, gb):
        """One 4-group block of a fused batch pair (108-col matmuls)."""
        if gb == 0:
            otps[p] = oppool.tile([P_OUT, G, 2, WO], bf16, name="otp")
            if p + 2 < NP:
                load_xp(p + 2)
        xc, ot = xpts[p], otps[p]
        pt = ppool.tile([P_OUT, 432], f32, name="pt")
        for gl in range(GPT_P):
            g = gb * GPT_P + gl
            for kw in range(KW):
                nc.tensor.matmul(
                    pt[:, gl * 2 * WO:(gl + 1) * 2 * WO],
                    lhsT=wt[:, g, kw, :],
                    rhs=xc[:, g, :, kw:kw + WO],
                    start=(kw == 0),
                    stop=(kw == KW - 1),
                )
        dst = ot[:, gb * GPT_P:(gb + 1) * GPT_P, :, :]
        if p == NP - 1 and gb == NGB_P - 1:
            # Final block: evict on the idle DVE sequencer and ship the
            # last 4 groups from the idle SP sequencer for a short drain.
            nc.vector.tensor_copy(dst, pt[:])
            nc.sync.dma_start(
                out=yp_d[p, :, G - GPT_P:, :, :],
                in_=ot[:, G - GPT_P:, :, :]
            )
        elif gb < NGB_P // 2:
            nc.vector.tensor_copy(dst, pt[:])
        else:
            nc.scalar.activation(
                dst, pt[:], mybir.ActivationFunctionType.Copy
            )
        # Ship y in quarters so each DMA waits on only 4 evictions
        # (more waits lower to long EventSemaphore chains that stall the
        # issuing sequencer and starve the DMA engines).
        Q = G // 4
        if gb == NGB_P // 4 - 1:
            nc.sync.dma_start(
                out=yp_d[p, :, :Q, :, :], in_=ot[:, :Q, :, :]
            )
        elif gb == NGB_P // 2 - 1:
            nc.sync.dma_start(
                out=yp_d[p, :, Q:2 * Q, :, :], in_=ot[:, Q:2 * Q, :, :]
            )
        elif gb == 3 * NGB_P // 4 - 1:
            nc.scalar.dma_start(
                out=yp_d[p, :, 2 * Q:3 * Q, :, :],
                in_=ot[:, 2 * Q:3 * Q, :, :]
            )
        elif gb == NGB_P - 2 and p == NP - 1:
            nc.scalar.dma_start(
                out=yp_d[p, :, 3 * Q:G - GPT_P, :, :],
                in_=ot[:, 3 * Q:G - GPT_P, :, :]
            )
        elif gb == NGB_P - 1:
            if p == NP - 1:
                pass  # groups 48..60 already shipped at gb14
            else:
                nc.scalar.dma_start(
                    out=yp_d[p, :, 3 * Q:, :, :], in_=ot[:, 3 * Q:, :, :]
                )
            xpts.pop(p)
            otps.pop(p)

    # Prologue: head batches 0-2 interleaved by block in release order,
    # then batch 3, then the fused pairs. The first two pair inputs are
    # issued mid-head so their 4.46us transfers slot in after the early
    # head outputs without starving them.
    PROLOGUE = [
        (0, 0), (0, 1), (1, 0), (1, 1), (0, 2), (1, 2),
        (2, 0), (2, 1), (2, 2),
    ] + [(n, gb) for gb in range(3, NGB_S) for n in range(3)]
    for j, (n, gb) in enumerate(PROLOGUE):
        emit_head(n, gb)
        if (n, gb) == (0, 3):
            load_xp(0)
        elif (n, gb) == (2, 3):
            load_xp(1)
    for gb in range(NGB_S):
        emit_head(3, gb)
    for p in range(NP):
        for gb in range(NGB_P):
            emit_pair(p, gb)


def _get_nc(repeats=1):
    if repeats not in _NC_CACHE:
        _NC_CACHE[repeats] = _build_program(repeats)
    return _NC_CACHE[repeats]


def _make_bands(w):
    """Per-core banded lhsT weights, shape (112, 64, 3, 108) bf16.

    bands[ic*56 + oy + kh, g, kw, oc*54 + oy] = w[2g+oc, ic, kh, kw]
    """
    w = np.asarray(w, dtype=np.float32)
    wg = w.reshape(G * N_CORES, 2, 2, KH, KW)  # [g_all, oc, ic, kh, kw]
    oy = np.arange(HO)
    mats = []
    for cid in range(N_CORES):
        ws = wg[cid * G:(cid + 1) * G]  # [G, oc, ic, kh, kw]
        bands = np.zeros((P_IN, G, KW, P_OUT), dtype=np.float32)
        for ic in range(2):
            for oc in range(2):
                for kh in range(KH):
                    bands[ic * H + oy + kh, :, :, oc * HO + oy] = (
                        ws[:, oc, ic, kh, :][None, :, :]
                    )
        mats.append(bands.astype(BF16))
    return mats


def _permute_x(x):
    """Full x -> per-core (x_head[n,(ic,dy),g,j], x_pairs[p,(ic,dy),g,nb,j])."""
    x = np.asarray(x)
    out = []
    for cid in range(N_CORES):
        xs = x[:, cid * CPC:(cid + 1) * CPC].astype(BF16)
        xg = xs.reshape(B, G, 2, H, W)
        xh = xg[:BH].transpose(0, 2, 3, 1, 4)  # [n, ic, dy, g, j]
        xp = xg[BH:].reshape(NP, 2, G, 2, H, W).transpose(0, 3, 4, 2, 1, 5)
        out.append((
            np.ascontiguousarray(xh.reshape(BH, P_IN, G, W)),
            np.ascontiguousarray(xp.reshape(NP, P_IN, G, 2, W)),
        ))
    return out


def _unpermute_y(res):
    """Per-core head+pair outputs -> full f32 NCHW."""
    parts = []
    for cid in range(N_CORES):
        yh = np.asarray(res[cid]["y"]).astype(np.float32)
        yp = np.asarray(res[cid]["yp"]).astype(np.float32)
        yhc = yh.reshape(BH, 2, HO, G, WO).transpose(0, 3, 1, 2, 4)
        ypc = yp.reshape(NP, 2, HO, G, 2, WO).transpose(0, 4, 3, 1, 2, 5)
        full = np.concatenate([
            yhc.reshape(BH, CPC, HO, WO),
            ypc.reshape(B - BH, CPC, HO, WO),
        ], axis=0)
        parts.append(full)
    return np.concatenate(parts, axis=1)


def _run(x, w, trace=False, **kwargs):
    nc = _get_nc()
    xperm = _permute_x(x)
    bands = _make_bands(w)
    in_maps = [
        {"x": xperm[cid][0], "xp": xperm[cid][1], "wb": bands[cid]}
        for cid in range(N_CORES)
    ]
    res = run_bass_kernel_spmd(
        nc, in_maps, list(range(N_CORES)), trace=trace, **kwargs
    )
    y = _unpermute_y(res.results)
    return y, res


def kernel(x, w):
    y, _ = _run(x, w, trace=False)
    return y


# revision 66
# speedup vs baseline: 2.2578x; 1.0051x over previous
"""Grouped Conv2d (512 groups, 2->2 ch/group, 3x3 VALID) on 8 trn2 NeuronCores.

Band-matrix formulation, bf16 end-to-end; batches 0-3 run singly (their
DMAs interleave with the banded-weight load), batches 4-15 run as fused
pairs with two batches in the matmul moving dim (108 columns), which
halves the matmul count and avoids the per-matmul ceil-to-ns rounding.
"""

import sys

import numpy as np

for _p in ("/opt/trn_rl_repo",):
    if _p not in sys.path:
        sys.path.insert(0, _p)

import ml_dtypes

import concourse.bacc as bacc
import concourse.bass as bass
import concourse.tile as tile
from concourse import mybir
from concourse.bass_utils import run_bass_kernel_spmd

N_CORES = 8
B, C, H, W = 16, 1024, 56, 56
BH = 4              # head batches, processed singly
NP = (B - BH) // 2  # 6 fused pairs
KH = KW = 3
HO, WO = H - KH + 1, W - KW + 1  # 54, 54
CPC = C // N_CORES  # 128 channels per core
G = CPC // 2  # 64 groups per core
P_IN = 2 * H  # 112 partitions: (ic, dy)
P_OUT = 2 * HO  # 108 lanes: (oc, oy)
GPT_S = 8  # groups per psum tile, single-batch blocks (8*54 = 432 fp32)
GPT_P = 4  # groups per psum tile, pair blocks (4*2*54 = 432 fp32)
NGB_S = G // GPT_S  # 8 blocks per head batch
NGB_P = G // GPT_P  # 16 blocks per pair

BF16 = ml_dtypes.bfloat16

_NC_CACHE = {}


def _build_program(repeats=1):
    nc = bacc.Bacc(
        "TRN2", target_bir_lowering=False, debug=False, num_devices=N_CORES
    )
    f32 = mybir.dt.float32
    bf16 = mybir.dt.bfloat16

    x_d = nc.declare_dram_parameter(
        "x", [BH, P_IN, G, W], bf16, isOutput=False
    )
    xp_d = nc.declare_dram_parameter(
        "xp", [NP, P_IN, G, 2, W], bf16, isOutput=False
    )
    wb_d = nc.declare_dram_parameter(
        "wb", [P_IN, G, KW, P_OUT], bf16, isOutput=False
    )
    y_d = nc.declare_dram_parameter(
        "y", [BH, P_OUT, G, WO], bf16, isOutput=True
    )
    yp_d = nc.declare_dram_parameter(
        "yp", [NP, P_OUT, G, 2, WO], bf16, isOutput=True
    )

    with tile.TileContext(nc) as tc:
        with (
            tc.tile_pool(name="wpool", bufs=1) as wpool,
            tc.tile_pool(name="xspool", bufs=4) as xspool,
            tc.tile_pool(name="xppool", bufs=3) as xppool,
            tc.tile_pool(name="ospool", bufs=4) as ospool,
            tc.tile_pool(name="oppool", bufs=3) as oppool,
            tc.tile_pool(name="psum", bufs=8, space="PSUM") as ppool,
        ):
            wt = wpool.tile([P_IN, G, KW, P_OUT], bf16)

            def body(first):
                _emit(nc, tc, xspool, xppool, ospool, oppool, ppool,
                      x_d, xp_d, y_d, yp_d, wb_d, wt, first)

            if repeats == 1:
                body(True)
            else:
                body(True)
                with tc.For_i(0, repeats - 1):
                    body(False)
    nc.compile()
    return nc


def _emit(nc, tc, xspool, xppool, ospool, oppool, ppool,
          x_d, xp_d, y_d, yp_d, wb_d, wt, first):
    f32 = mybir.dt.float32
    bf16 = mybir.dt.bfloat16

    xts, xpts = {}, {}

    def load_x(n):
        xts[n] = xspool.tile([P_IN, G, W], bf16, name="xt")
        nc.sync.dma_start(out=xts[n][:], in_=x_d[n])

    def load_xp(p):
        xpts[p] = xppool.tile([P_IN, G, 2, W], bf16, name="xtp")
        nc.sync.dma_start(out=xpts[p][:], in_=xp_d[p])

    if first:
        # Weight chunks pace the prologue; the head-batch inputs
        # interleave with the early chunks so the 12.9us weight load
        # hides behind batches 0-2's compute.
        WCHUNK = 4
        for gc in range(G // WCHUNK):
            lo, hi = gc * WCHUNK, (gc + 1) * WCHUNK
            nc.sync.dma_start(out=wt[:, lo:hi], in_=wb_d[:, lo:hi])
            if gc in (0, 2, 4):
                load_x(gc // 2)
        load_x(3)

        # Dummy matmuls ramp the PE clock gate while the weight and
        # first-input DMAs drain.
        for s in range(8):
            scr = ppool.tile([P_OUT, 432], f32, name="pt")
            for _ in range(2 if s < 4 else 1):
                nc.tensor.matmul(
                    scr[:, :KW * P_OUT], lhsT=wt[:, 0, 0, :],
                    rhs=wt[:, 0, :, :], start=True, stop=True,
                )
    else:
        for k in range(BH):
            load_x(k)

    ots, otps = {}, {}

    def emit_head(n, gb):
        """One 8-group block of a single head batch."""
        if gb == 0:
            ots[n] = ospool.tile([P_OUT, G, WO], bf16, name="ot")
        xc, ot = xts[n], ots[n]
        pt = ppool.tile([P_OUT, 432], f32, name="pt")
        for gl in range(GPT_S):
            g = gb * GPT_S + gl
            for kw in range(KW):
                nc.tensor.matmul(
                    pt[:, gl * WO:(gl + 1) * WO],
                    lhsT=wt[:, g, kw, :],
                    rhs=xc[:, g, kw:kw + WO],
                    start=(kw == 0),
                    stop=(kw == KW - 1),
                )
        dst = ot[:, gb * GPT_S:(gb + 1) * GPT_S, :]
        if gb < NGB_S // 2:
            nc.vector.tensor_copy(dst, pt[:])
        else:
            nc.scalar.activation(
                dst, pt[:], mybir.ActivationFunctionType.Copy
            )
        if gb == NGB_S // 2 - 1:
            nc.sync.dma_start(
                out=y_d[n, :, :G // 2, :], in_=ot[:, :G // 2, :]
            )
        elif gb == NGB_S - 1:
            nc.scalar.dma_start(
                out=y_d[n, :, G // 2:, :], in_=ot[:, G // 2:, :]
            )
            xts.pop(n)
            ots.pop(n)

    def emit_pair(p, gb):
        """One 4-group block of a fused batch pair (108-col matmuls)."""
        if gb == 0:
            otps[p] = oppool.tile([P_OUT, G, 2, WO], bf16, name="otp")
            if p + 2 < NP:
                load_xp(p + 2)
        xc, ot = xpts[p], otps[p]
        pt = ppool.tile([P_OUT, 432], f32, name="pt")
        for gl in range(GPT_P):
            g = gb * GPT_P + gl
            for kw in range(KW):
                nc.tensor.matmul(
                    pt[:, gl * 2 * WO:(gl + 1) * 2 * WO],
                    lhsT=wt[:, g, kw, :],
                    rhs=xc[:, g, :, kw:kw + WO],
                    start=(kw == 0),
                    stop=(kw == KW - 1),
                )
        dst = ot[:, gb * GPT_P:(gb + 1) * GPT_P, :, :]
        if p == NP - 1 and gb == NGB_P - 1:
            # Final block: evict on the idle DVE sequencer and ship the
            # last 4 groups from the idle SP sequencer for a short drain.
            nc.vector.tensor_copy(dst, pt[:])
            nc.sync.dma_start(
                out=yp_d[p, :, G - 2 * GPT_P:, :, :],
                in_=ot[:, G - 2 * GPT_P:, :, :]
            )
        elif gb < NGB_P // 2:
            nc.vector.tensor_copy(dst, pt[:])
        else:
            nc.scalar.activation(
                dst, pt[:], mybir.ActivationFunctionType.Copy
            )
        # Ship y in quarters so each DMA waits on only 4 evictions
        # (more waits lower to long EventSemaphore chains that stall the
        # issuing sequencer and starve the DMA engines).
        Q = G // 4
        if gb == NGB_P // 4 - 1:
            nc.sync.dma_start(
                out=yp_d[p, :, :Q, :, :], in_=ot[:, :Q, :, :]
            )
        elif gb == NGB_P // 2 - 1:
            nc.sync.dma_start(
                out=yp_d[p, :, Q:2 * Q, :, :], in_=ot[:, Q:2 * Q, :, :]
            )
        elif gb == 3 * NGB_P // 4 - 1:
            nc.scalar.dma_start(
                out=yp_d[p, :, 2 * Q:3 * Q, :, :],
                in_=ot[:, 2 * Q:3 * Q, :, :]
            )
        elif gb == NGB_P - 2 and p == NP - 1:
            nc.scalar.dma_start(
                out=yp_d[p, :, 3 * Q:3 * Q + 8, :, :],
                in_=ot[:, 3 * Q:3 * Q + 8, :, :]
            )
        elif gb == NGB_P - 1:
            if p == NP - 1:
                pass  # groups 48..60 already shipped at gb14
            else:
                nc.scalar.dma_start(
                    out=yp_d[p, :, 3 * Q:, :, :], in_=ot[:, 3 * Q:, :, :]
                )
            xpts.pop(p)
            otps.pop(p)

    # Prologue: head batches 0-2 interleaved by block in release order,
    # then batch 3, then the fused pairs. The first two pair inputs are
    # issued mid-head so their 4.46us transfers slot in after the early
    # head outputs without starving them.
    PROLOGUE = [
        (0, 0), (0, 1), (1, 0), (1, 1), (0, 2), (1, 2),
        (2, 0), (2, 1), (2, 2),
    ] + [(n, gb) for gb in range(3, NGB_S) for n in range(3)]
    for j, (n, gb) in enumerate(PROLOGUE):
        emit_head(n, gb)
        if (n, gb) == (0, 3):
            load_xp(0)
        elif (n, gb) == (2, 3):
            load_xp(1)
    for gb in range(NGB_S):
        emit_head(3, gb)
    for p in range(NP):
        for gb in range(NGB_P):
            emit_pair(p, gb)


def _get_nc(repeats=1):
    if repeats not in _NC_CACHE:
        _NC_CACHE[repeats] = _build_program(repeats)
    return _NC_CACHE[repeats]


def _make_bands(w):
    """Per-core banded lhsT weights, shape (112, 64, 3, 108) bf16.

    bands[ic*56 + oy + kh, g, kw, oc*54 + oy] = w[2g+oc, ic, kh, kw]
    """
    w = np.asarray(w, dtype=np.float32)
    wg = w.reshape(G * N_CORES, 2, 2, KH, KW)  # [g_all, oc, ic, kh, kw]
    oy = np.arange(HO)
    mats = []
    for cid in range(N_CORES):
        ws = wg[cid * G:(cid + 1) * G]  # [G, oc, ic, kh, kw]
        bands = np.zeros((P_IN, G, KW, P_OUT), dtype=np.float32)
        for ic in range(2):
            for oc in range(2):
                for kh in range(KH):
                    bands[ic * H + oy + kh, :, :, oc * HO + oy] = (
                        ws[:, oc, ic, kh, :][None, :, :]
                    )
        mats.append(bands.astype(BF16))
    return mats


def _permute_x(x):
    """Full x -> per-core (x_head[n,(ic,dy),g,j], x_pairs[p,(ic,dy),g,nb,j])."""
    x = np.asarray(x)
    out = []
    for cid in range(N_CORES):
        xs = x[:, cid * CPC:(cid + 1) * CPC].astype(BF16)
        xg = xs.reshape(B, G, 2, H, W)
        xh = xg[:BH].transpose(0, 2, 3, 1, 4)  # [n, ic, dy, g, j]
        xp = xg[BH:].reshape(NP, 2, G, 2, H, W).transpose(0, 3, 4, 2, 1, 5)
        out.append((
            np.ascontiguousarray(xh.reshape(BH, P_IN, G, W)),
            np.ascontiguousarray(xp.reshape(NP, P_IN, G, 2, W)),
        ))
    return out


def _unpermute_y(res):
    """Per-core head+pair outputs -> full f32 NCHW."""
    parts = []
    for cid in range(N_CORES):
        yh = np.asarray(res[cid]["y"]).astype(np.float32)
        yp = np.asarray(res[cid]["yp"]).astype(np.float32)
        yhc = yh.reshape(BH, 2, HO, G, WO).transpose(0, 3, 1, 2, 4)
        ypc = yp.reshape(NP, 2, HO, G, 2, WO).transpose(0, 4, 3, 1, 2, 5)
        full = np.concatenate([
            yhc.reshape(BH, CPC, HO, WO),
            ypc.reshape(B - BH, CPC, HO, WO),
        ], axis=0)
        parts.append(full)
    return np.concatenate(parts, axis=1)


def _run(x, w, trace=False, **kwargs):
    nc = _get_nc()
    xperm = _permute_x(x)
    bands = _make_bands(w)
    in_maps = [
        {"x": xperm[cid][0], "xp": xperm[cid][1], "wb": bands[cid]}
        for cid in range(N_CORES)
    ]
    res = run_bass_kernel_spmd(
        nc, in_maps, list(range(N_CORES)), trace=trace, **kwargs
    )
    y = _unpermute_y(res.results)
    return y, res


def kernel(x, w):
    y, _ = _run(x, w, trace=False)
    return y
